# revision 48
# baseline (speedup 1.0000x reference)
"""Causal attention with memory + post-softmax expire gating, on 8 trn2 cores.

Sharding: batch (2) x head-groups (4 heads each) -> 8 cores. Each core
computes q/k/v projections for its 4 heads (column-parallel), local
attention, and a partial output projection (row-parallel over heads).
Host sums the 4 partial products per batch and adds the bias.

v3: software-pipelined emission.
  - softmax denominator rides the PV matmul as a 65th stationary column.
  - ctx DMA: q's it0 column chunk first, then mem half, then the rest.
  - projections / output projection spread one item per jj iteration.
  - PV lags S by one jj so PE has independent work while ACT drains.

v12 (~356us, from 419us):
  - DMAs split across sync+scalar HWDGE queues (~268GB/s each), per-db
    rings so projection db-accumulation paces with chunk arrivals.
  - warm-up matmuls cover the DMA-bound start so the HAM clock-gate
    (1.2->2.4GHz) warms early and never re-gates (throttle 96us -> 27us).
  - minimal prologue (q00/k00 only; q01/k01/v* ride the pending queue):
    first exp at ~29us instead of ~61us.
  - finalize chain: ACT-copy denominator rows (no table switch), bf16
    ones-broadcast matmul, one full-lane reciprocal_approx_fast --
    replaces 16x 3.3us single-lane DVE reciprocals; kills the ~9us
    PE stall + HAM re-gate at every i-block boundary.
  - diagonal tiles: S/exp/mask/PV restricted to live columns.
  - k-projection filler between carry-PV and finalizes at boundaries.
  - tail: per-ib out-DMAs, copies split ACT||DVE.

v13 (~355us): finalize lsb copies moved ACT->DVE (ACT gates the late
  jjs of it1-3 where per-jj exp 4.6us > per-jj S+PV 3.1us); pending
  pops spread (2/section for jj<2, then 1) so filler lasts the loop.
  Measured (NTFF, min-of-3; board clock varies ~20% run-to-run):
  PE busy ~326us (gapless), ACT ~222us, DVE ~92us.
  Also dead: bf16 output partials (DMA halves but cast/granularity
  costs more, +0.2e-3 err for +3us).
  Dead ends (hardware-measured): PV col-tile head-packing pairs stream
  1.71x but separate denominator matmuls eat the entire gain; M=1
  denom quads ~99ns/MM still net-negative.
"""

import numpy as np
import ml_dtypes
from contextlib import ExitStack

import concourse.bass as bass
import concourse.mybir as mybir
import concourse.tile as tile
from concourse import bacc
from concourse.bass_utils import run_bass_kernel_spmd

F32 = mybir.dt.float32
BF16 = mybir.dt.bfloat16
AF = mybir.ActivationFunctionType
MULT = mybir.AluOpType.mult

HEADS = 16
B, N, MEM, DIM = 2, 2048, 2048, 1024
J = MEM + N                      # 4096
DH = 64                          # head dim
HPC = 4                          # heads per core
DHC = HPC * DH                   # 256 dims per core
SCALE = DH ** -0.5
NCORES = 8

NJB = J // 128                   # 32 j-blocks
NIT = N // 512                   # 4 i-blocks
NDB = DIM // 128                 # 8 D-blocks

REPS = 1                         # test-only: on-device repeat count for timing
UNROLL = False                   # test-only: python-unroll reps (for TimelineSim)


def build_program_v(reps=1, unroll=False):
    global REPS, UNROLL
    old = (REPS, UNROLL)
    REPS, UNROLL = reps, unroll
    try:
        return build_program()
    finally:
        REPS, UNROLL = old


def _njb(it):
    return 4 * it + 20


def _off(it, jb):
    return 128 * jb - MEM - 512 * it


def build_program():
    nc = bacc.Bacc("TRN2", target_bir_lowering=False, debug=False,
                   num_devices=NCORES)
    ctxT_d = nc.dram_tensor("ctxT", [DIM, J], BF16, kind="ExternalInput").ap()
    wq_d = nc.dram_tensor("wq", [DIM, DHC], BF16, kind="ExternalInput").ap()
    wk_d = nc.dram_tensor("wk", [DIM, DHC], BF16, kind="ExternalInput").ap()
    wv_d = nc.dram_tensor("wv", [DIM, DHC], BF16, kind="ExternalInput").ap()
    wo_d = nc.dram_tensor("wo", [DHC, DIM], BF16, kind="ExternalInput").ap()
    exp_d = nc.dram_tensor("expire", [NJB, 128], F32, kind="ExternalInput").ap()
    msk_d = nc.dram_tensor("masks", [4, 128, 512], BF16, kind="ExternalInput").ap()
    out_d = nc.dram_tensor("out", [N, DIM], F32, kind="ExternalOutput").ap()

    with tile.TileContext(nc) as tc, ExitStack() as ctx:
        sb = ctx.enter_context(tc.tile_pool(name="sb", bufs=1))
        pb = ctx.enter_context(tc.tile_pool(name="pb", bufs=1))
        ob = ctx.enter_context(tc.tile_pool(name="ob", bufs=1))
        pp = ctx.enter_context(tc.tile_pool(name="pp", bufs=1, space="PSUM"))
        dp = ctx.enter_context(tc.tile_pool(name="dp", bufs=2, space="DRAM"))

        # ---- constants / small inputs ----
        expire = sb.tile([128, NJB], F32)
        masks = sb.tile([128, 4, 512], BF16)
        wq = sb.tile([128, NDB, DHC], BF16)
        wk = sb.tile([128, NDB, DHC], BF16)
        wv = sb.tile([128, NDB, DHC], BF16)
        cx = sb.tile([128, NDB, J], BF16)
        wo = sb.tile([128, 2, DIM], BF16)

        # DMA in first-use order, split across the two HWDGE queues (sync +
        # scalar) so descriptor-ring issue (~0.8us each) doesn't serialize:
        # sync: wq + it0 q-chunk (unblocks the first matmul), k/v weights.
        # scalar (idle until the first exp): the big ctx waves, masks, wo.
        # pr0 halves first: q00/k00 need only columns 0:128 of wq/wk, so
        # the q-chunk/mem waves start ~1us earlier; pr1 halves follow the
        # first ctx wave (consumed via pending pops much later)
        nc.sync.dma_start(out=wq[:, :, 0:128],
                          in_=wq_d[:, 0:128].rearrange("(db p) m -> p db m", p=128))
        nc.scalar.dma_start(out=wk[:, :, 0:128],
                            in_=wk_d[:, 0:128].rearrange("(db p) m -> p db m", p=128))

        # augmented v': per head 64 v-dims * expire + ones column (denom)
        ones64 = sb.tile([1, 64], BF16)
        nc.vector.memset(ones64, 1.0)
        warm_r = sb.tile([1, 512], BF16)
        nc.vector.memset(warm_r, 1.0)
        vpa = sb.tile([128, NJB, HPC, DH + 1], BF16)
        nc.vector.memset(vpa[:, :, :, DH:DH + 1], 1.0)

        qT = [sb.tile([128, N], BF16, name=f"qT{p}", tag=f"qT{p}") for p in range(2)]
        kT = [sb.tile([128, J], BF16, name=f"kT{p}", tag=f"kT{p}") for p in range(2)]
        ao = [sb.tile([128, N], BF16, name=f"ao{p}", tag=f"ao{p}") for p in range(2)]

        # keep the PE busy while the first DMAs land so the HAM clock-gate
        # reaches 8/8 before the first real matmul (and stays there)
        warm_ps = pp.tile([128, 1024], F32, name="warm", tag="s", bufs=2)

        def warm(n, cols=128):
            for _w in range(n):
                nc.tensor.matmul(warm_ps[0:64, 0:cols], lhsT=ones64,
                                 rhs=warm_r[:, 0:cols],
                                 start=True, stop=True, skip_group_check=True)

        warm(40)

        rep_cm = tc.For_i(0, REPS, 1) if REPS > 1 and not UNROLL else None
        if rep_cm is not None:
            rep_cm.__enter__()

        # ---- context load, ordered by first use and split across the two
        # HWDGE queues (~268GB/s each). sync: q-chunk, wv, mem jt2/3, x rest;
        # scalar: mem jt0/jt1 (rings done before the first exp needs ACT).
        for _rep in range(REPS if UNROLL else 1):
            for db in range(NDB):
                nc.sync.dma_start(out=cx[:, db, MEM:MEM + 512],
                                  in_=ctxT_d[128 * db:128 * db + 128, MEM:MEM + 512])
            for db in range(NDB):
                nc.scalar.dma_start(out=cx[:, db, 0:512],
                                    in_=ctxT_d[128 * db:128 * db + 128, 0:512])
            nc.sync.dma_start(out=wq[:, :, 128:256],
                              in_=wq_d[:, 128:256].rearrange("(db p) m -> p db m",
                                                             p=128))
            nc.sync.dma_start(out=wv, in_=wv_d.rearrange("(db p) m -> p db m", p=128))
            nc.sync.dma_start(out=expire, in_=exp_d.rearrange("j p -> p j"))
            nc.scalar.dma_start(out=wk[:, :, 128:256],
                                in_=wk_d[:, 128:256].rearrange("(db p) m -> p db m",
                                                               p=128))
            for db in range(NDB):
                nc.scalar.dma_start(out=cx[:, db, 512:1024],
                                    in_=ctxT_d[128 * db:128 * db + 128, 512:1024])
            for db in range(NDB):
                nc.sync.dma_start(out=cx[:, db, 1024:MEM],
                                  in_=ctxT_d[128 * db:128 * db + 128, 1024:MEM])
            nc.sync.dma_start(out=masks, in_=msk_d.rearrange("o p i -> p o i"))
            nc.sync.dma_start(out=wo, in_=wo_d.rearrange("(pr p) m -> p pr m", p=128))
            for db in range(NDB):
                nc.sync.dma_start(out=cx[:, db, MEM + 512:J],
                                  in_=ctxT_d[128 * db:128 * db + 128, MEM + 512:J])

            def proj_k(jt, pr):
                ps = pp.tile([128, 1024], F32, name="ps", tag="s", bufs=2)
                for db in range(NDB):
                    nc.tensor.matmul(
                        ps[:, 0:512], lhsT=wk[:, db, 128 * pr:128 * pr + 128],
                        rhs=cx[:, db, 512 * jt:512 * jt + 512],
                        start=(db == 0), stop=(db == NDB - 1))
                nc.vector.tensor_copy(out=kT[pr][:, 512 * jt:512 * jt + 512],
                                      in_=ps[:, 0:512])

            def proj_q(it, pr):
                ps = pp.tile([128, 1024], F32, name="ps", tag="s", bufs=2)
                for db in range(NDB):
                    nc.tensor.matmul(
                        ps[:, 0:512], lhsT=wq[:, db, 128 * pr:128 * pr + 128],
                        rhs=cx[:, db, MEM + 512 * it:MEM + 512 * it + 512],
                        start=(db == 0), stop=(db == NDB - 1))
                nc.vector.tensor_copy(out=qT[pr][:, 512 * it:512 * it + 512],
                                      in_=ps[:, 0:512])

            def proj_v(jb):
                ps = pp.tile([128, 1024], F32, name="ps", tag="s", bufs=2)
                for db in range(NDB):
                    nc.tensor.matmul(
                        ps[:, 0:DHC], lhsT=cx[:, db, 128 * jb:128 * jb + 128],
                        rhs=wv[:, db, :],
                        start=(db == 0), stop=(db == NDB - 1))
                nc.vector.tensor_scalar(
                    out=vpa[:, jb, :, 0:DH],
                    in0=ps[:, 0:DHC].rearrange("p (h d) -> p h d", h=HPC),
                    scalar1=expire[:, jb:jb + 1], scalar2=None, op0=MULT)

            ot_hold = {}

            def outproj_ib(ib):
                # one matmul+copy per ib; quad out-DMA fires on ib%4==3
                ps = pp.tile([128, 1024], F32, name="ps_o", tag="s", bufs=2)
                for nb in range(2):
                    for pr in range(2):
                        nc.tensor.matmul(
                            ps[:, 512 * nb:512 * nb + 512],
                            lhsT=ao[pr][:, 128 * ib:128 * ib + 128],
                            rhs=wo[:, pr, 512 * nb:512 * nb + 512],
                            start=(pr == 0), stop=(pr == 1))
                half = ib % 4
                if half == 0:
                    ot_hold["t"] = ob.tile([128, 4, 1024], F32, name="ot",
                                           tag="ot", bufs=2)
                ot4 = ot_hold["t"]
                nc.vector.tensor_copy(out=ot4[:, half, :], in_=ps)
                if half == 3:
                    ib0 = ib - 3
                    nc.sync.dma_start(
                        out=out_d[128 * ib0:128 * ib0 + 512, :].rearrange(
                            "(i p) n -> p i n", p=128),
                        in_=ot4)

            def emit_pv_group(pvd, prevmap, is_first, is_last, heads):
                pv = pvd
                for h in heads:
                    p_t, jb0, pit = prevmap[h]
                    for half, jb in enumerate((jb0, jb0 + 1)):
                        off = _off(pit, jb)
                        lo = off if 0 <= off < 512 else 0
                        nc.tensor.matmul(
                            pv[h][0:DH + 1, lo:512],
                            lhsT=vpa[:, jb, h, :],
                            rhs=p_t[:, 512 * half + lo:512 * half + 512],
                            start=(is_first and half == 0),
                            stop=(is_last and half == 1),
                            skip_group_check=True)

            def finalize_pr(pvd, isl, pr):
                # denominator rows (bf16, ACT copy: PSUM-close, no table
                # switch) -> broadcast l to 64 rows via contract-1 PE matmul
                # -> one full-lane approx reciprocal -> scale.
                pv = pvd
                lsb = [ob.tile([1, 512], BF16, name=f"lsb{e}", tag=f"lsb{e}",
                               bufs=2) for e in range(2)]
                with nc.allow_low_precision(reason="1/l broadcast in bf16"):
                    for e in range(2):
                        h = 2 * pr + e
                        nc.vector.tensor_copy(out=lsb[e],
                                              in_=pv[h][DH:DH + 1, :])
                bc_ps = pp.tile([128, 1024], F32, name="bc_ps", tag="s", bufs=2)
                for e in range(2):
                    nc.tensor.matmul(bc_ps[64 * e:64 * e + 64, 0:512],
                                     lhsT=ones64, rhs=lsb[e],
                                     start=True, stop=True,
                                     tile_position=(0, 64 * e),
                                     skip_group_check=True)
                bc = ob.tile([128, 512], F32, name="bc", tag="bc", bufs=2)
                nc.vector.reciprocal_approx_fast(out=bc, in_=bc_ps[:, 0:512])
                for e in range(2):
                    h = 2 * pr + e
                    nc.vector.tensor_tensor(ao[pr][64 * e:64 * e + 64, isl],
                                            pv[h][0:DH, :],
                                            bc[64 * e:64 * e + 64, :], MULT)

            def run_pending(pending, n=2):
                for _ in range(min(n, len(pending))):
                    kind, arg = pending.pop(0)
                    if kind == "k":
                        proj_k(*arg)
                    elif kind == "v":
                        proj_v(arg)
                    elif kind == "q":
                        proj_q(*arg)
                    elif kind == "o":
                        outproj_ib(arg)
                    elif kind == "fp":
                        emit_pv_group(*arg)
                    elif kind == "fin":
                        finalize_pr(*arg)
                    elif kind == "w":
                        warm(arg, 512)

            # ---- prologue: bare minimum for S at it0 jj0 pr0; pr1's q/k
            # are the first pending pops (emitted between pr0 and pr1
            # sections of jj0), so the first exp starts ~10us earlier.
            proj_q(0, 0)
            proj_k(0, 0)

            carry = None   # prev it's (pv, prevmap, first_flag, isl) awaiting PV+finalize
            for it in range(NIT):
                njb = _njb(it)
                npair = njb // 2
                i0 = 512 * it
                isl = slice(i0, i0 + 512)

                pending = []
                if carry is not None:
                    # real PE work (k-proj) between the carry PV and each
                    # finalize fills the ACT-copy/approx latency; the "w"
                    # bundles are backstop so the HAM clock never re-gates
                    cpv, cprev, cfirst, cisl = carry
                    pending += [("fp", (cpv, cprev, cfirst, True, range(HPC))),
                                ("k", (4 + it, 0)),
                                ("fin", (cpv, cisl, 0)),
                                ("k", (4 + it, 1)),
                                ("fin", (cpv, cisl, 1))]
                if it == 0:
                    # deadlines: k jt (both prs) before S at jj=2*jt; v jb
                    # before PV at jj=jb//2+1 (PV lags one jj). Four pops per
                    # jj (two per pr section) meet these comfortably. q(1) is
                    # last (x-chunk DMA lands late).
                    pending += [("q", (0, 1)), ("k", (0, 1)),
                                ("v", 0), ("v", 1), ("v", 2), ("v", 3)]
                    for jt in range(1, 5):
                        pending += [("k", (jt, 0)), ("k", (jt, 1)),
                                    ("v", 2 * jt + 2), ("v", 2 * jt + 3)]
                    pending += [("v", jb) for jb in range(12, 20)]
                    pending += [("q", (1, 0)), ("q", (1, 1))]
                else:
                    pending += [("v", 16 + 4 * it), ("v", 17 + 4 * it),
                                ("v", 18 + 4 * it), ("v", 19 + 4 * it)]
                    if it < NIT - 1:
                        pending += [("q", (it + 1, 0)), ("q", (it + 1, 1))]
                    pending += [("o", ib) for ib in range(4 * (it - 1), 4 * it)]

                pvd = [pp.tile([128, 512], F32, name=f"pv{h}", tag=f"pv{h}",
                               bufs=1) for h in range(HPC)]

                prev = None
                for jj in range(npair):
                    jb0 = 2 * jj
                    # diagonal tiles: columns below `off` are fully masked —
                    # S/exp/mask/PV all restrict to the live column range
                    off0 = _off(it, jb0)
                    lo0 = off0 if 0 <= off0 < 512 else 0
                    cur = {}
                    for pr in range(2):
                        s_h = [pp.tile([128, 1024], F32, name=f"s{e}", tag="s",
                                       bufs=2) for e in range(2)]
                        for half, jb in enumerate((jb0, jb0 + 1)):
                            off = _off(it, jb)
                            lo = off if 0 <= off < 512 else 0
                            jsl = slice(128 * jb, 128 * jb + 128)
                            fsl = slice(512 * half + lo, 512 * half + 512)
                            qsl = slice(i0 + lo, i0 + 512)
                            nc.tensor.matmul(s_h[0][:, fsl], lhsT=kT[pr][0:64, jsl],
                                             rhs=qT[pr][0:64, qsl],
                                             start=True, stop=True, tile_position=(0, 0))
                            nc.tensor.matmul(s_h[1][:, fsl], lhsT=kT[pr][64:128, jsl],
                                             rhs=qT[pr][64:128, qsl],
                                             start=True, stop=True, tile_position=(64, 0))
                        for e in range(2):
                            h = 2 * pr + e
                            p_t = pb.tile([128, 1024], BF16, name="p_t", tag="p", bufs=8)
                            nc.scalar.activation(p_t[:, lo0:1024], s_h[e][:, lo0:1024],
                                                 AF.Exp, scale=SCALE)
                            for half, jb in enumerate((jb0, jb0 + 1)):
                                off = _off(it, jb)
                                if 0 <= off < 512:
                                    msl = slice(512 * half + off,
                                                512 * half + off + 128)
                                    nc.vector.tensor_tensor(
                                        p_t[:, msl], p_t[:, msl],
                                        masks[:, off // 128, off:off + 128], MULT)
                            cur[h] = (p_t, jb0, it)
                        # after S of this pr: PV of previous jj (same heads)
                        if prev is not None:
                            emit_pv_group(pvd, prev, jj == 1, False,
                                          (2 * pr, 2 * pr + 1))
                        if it == 0 or jj < 2:
                            run_pending(pending, 2)
                        elif pr == 0:
                            run_pending(pending, 1)
                    prev = cur
                run_pending(pending, len(pending))
                carry = (pvd, prev, npair == 1, isl)

            # last it: pr1's PV+finalize first so its outproj partials can
            # start while pr0's finalize chain drains; copies split across
            # ACT+DVE; per-ib DMAs fire as each copy lands
            cpv, cprev, cfirst, cisl = carry
            emit_pv_group(cpv, cprev, cfirst, True, range(HPC))
            finalize_pr(cpv, cisl, 0)
            finalize_pr(cpv, cisl, 1)

            qb = 4 * (NIT - 1)
            ot4 = ob.tile([128, 4, 1024], F32, name="ot", tag="ot", bufs=2)
            for p2 in range(2):
                pss = []
                for ib in (qb + 2 * p2, qb + 2 * p2 + 1):
                    ps = pp.tile([128, 1024], F32, name="ps_o", tag="s", bufs=2)
                    pss.append((ib, ps))
                for pr in range(2):
                    for ib, ps in pss:
                        for nb in range(2):
                            nc.tensor.matmul(
                                ps[:, 512 * nb:512 * nb + 512],
                                lhsT=ao[pr][:, 128 * ib:128 * ib + 128],
                                rhs=wo[:, pr, 512 * nb:512 * nb + 512],
                                start=(pr == 0), stop=(pr == 1))
                for idx, (ib, ps) in enumerate(pss):
                    if idx == 0:
                        nc.scalar.activation(ot4[:, ib - qb, :], ps, AF.Copy)
                    else:
                        nc.vector.tensor_copy(out=ot4[:, ib - qb, :], in_=ps)
                    # alternate HWDGE queues so the four 512KB tail
                    # transfers run two-abreast instead of serializing
                    eng = nc.sync if idx == 0 else nc.scalar
                    eng.dma_start(
                        out=out_d[128 * ib:128 * ib + 128, :],
                        in_=ot4[:, ib - qb, :])
        if rep_cm is not None:
            rep_cm.__exit__(None, None, None)
    nc.compile()
    return nc


_NC = None


def _get_nc():
    global _NC
    if _NC is None:
        _NC = build_program()
    return _NC


def _make_masks():
    m = np.zeros((4, 128, 512), dtype=ml_dtypes.bfloat16)
    fi = np.arange(512)[None, :]
    fj = np.arange(128)[:, None]
    for o in range(4):
        m[o] = (fi >= fj + 128 * o).astype(ml_dtypes.bfloat16)
    return m


def make_in_maps(x, mem, expire_mask, Wq, Wkv, Wo):
    bf = ml_dtypes.bfloat16
    masks = _make_masks()
    ctxT = []
    for b in range(B):
        c = np.concatenate([mem[b], x[b]], axis=0)          # [J, DIM]
        ctxT.append(np.ascontiguousarray(c.T).astype(bf))   # [DIM, J]

    in_maps = []
    for core in range(NCORES):
        b, hg = core // 4, core % 4
        cs = slice(DHC * hg, DHC * hg + DHC)
        in_maps.append({
            "ctxT": ctxT[b],
            "wq": np.ascontiguousarray(Wq[:, cs]).astype(bf),
            "wk": np.ascontiguousarray(Wkv[:, cs]).astype(bf),
            "wv": np.ascontiguousarray(Wkv[:, DIM + cs.start:DIM + cs.stop]).astype(bf),
            "wo": np.ascontiguousarray(Wo[cs, :]).astype(bf),
            "expire": np.ascontiguousarray(expire_mask[b, 0, 0].reshape(NJB, 128)),
            "masks": masks,
        })
    return in_maps


def kernel(x, mem, expire_mask, Wq, Wkv, Wo, bo):
    x = np.asarray(x, dtype=np.float32)
    mem = np.asarray(mem, dtype=np.float32)
    expire_mask = np.asarray(expire_mask, dtype=np.float32)
    Wq = np.asarray(Wq, dtype=np.float32)
    Wkv = np.asarray(Wkv, dtype=np.float32)
    Wo = np.asarray(Wo, dtype=np.float32)
    bo = np.asarray(bo, dtype=np.float32)

    in_maps = make_in_maps(x, mem, expire_mask, Wq, Wkv, Wo)
    nc = _get_nc()
    res = run_bass_kernel_spmd(nc, in_maps, core_ids=list(range(NCORES)))

    out = np.zeros((B, N, DIM), dtype=np.float32)
    for core in range(NCORES):
        out[core // 4] += res.results[core]["out"].astype(np.float32)
    out += bo[None, None, :]
    return out



# revision 49
# speedup vs baseline: 1.1848x; 1.1848x over previous
"""Causal attention with memory + post-softmax expire gating, on 8 trn2 cores.

Sharding: batch (2) x head-groups (4 heads each) -> 8 cores. Each core
computes q/k/v projections for its 4 heads (column-parallel), local
attention, and a partial output projection (row-parallel over heads).
Host sums the 4 partial products per batch and adds the bias.

v3: software-pipelined emission.
  - softmax denominator rides the PV matmul as a 65th stationary column.
  - ctx DMA: q's it0 column chunk first, then mem half, then the rest.
  - projections / output projection spread one item per jj iteration.
  - PV lags S by one jj so PE has independent work while ACT drains.

v12 (~356us, from 419us):
  - DMAs split across sync+scalar HWDGE queues (~268GB/s each), per-db
    rings so projection db-accumulation paces with chunk arrivals.
  - warm-up matmuls cover the DMA-bound start so the HAM clock-gate
    (1.2->2.4GHz) warms early and never re-gates (throttle 96us -> 27us).
  - minimal prologue (q00/k00 only; q01/k01/v* ride the pending queue):
    first exp at ~29us instead of ~61us.
  - finalize chain: ACT-copy denominator rows (no table switch), bf16
    ones-broadcast matmul, one full-lane reciprocal_approx_fast --
    replaces 16x 3.3us single-lane DVE reciprocals; kills the ~9us
    PE stall + HAM re-gate at every i-block boundary.
  - diagonal tiles: S/exp/mask/PV restricted to live columns.
  - k-projection filler between carry-PV and finalizes at boundaries.
  - tail: per-ib out-DMAs, copies split ACT||DVE.

v13 (~355us): finalize lsb copies moved ACT->DVE (ACT gates the late
  jjs of it1-3 where per-jj exp 4.6us > per-jj S+PV 3.1us); pending
  pops spread (2/section for jj<2, then 1) so filler lasts the loop.
  Measured (NTFF, min-of-3; board clock varies ~20% run-to-run):
  PE busy ~326us (gapless), ACT ~222us, DVE ~92us.
  Also dead: bf16 output partials (DMA halves but cast/granularity
  costs more, +0.2e-3 err for +3us).
  Dead ends (hardware-measured): PV col-tile head-packing pairs stream
  1.71x but separate denominator matmuls eat the entire gain; M=1
  denom quads ~99ns/MM still net-negative.
"""

import numpy as np
import ml_dtypes
from contextlib import ExitStack

import concourse.bass as bass
import concourse.mybir as mybir
import concourse.tile as tile
from concourse import bacc
from concourse.bass_utils import run_bass_kernel_spmd

F32 = mybir.dt.float32
BF16 = mybir.dt.bfloat16
AF = mybir.ActivationFunctionType
MULT = mybir.AluOpType.mult

HEADS = 16
B, N, MEM, DIM = 2, 2048, 2048, 1024
J = MEM + N                      # 4096
DH = 64                          # head dim
HPC = 4                          # heads per core
DHC = HPC * DH                   # 256 dims per core
SCALE = DH ** -0.5
NCORES = 8

NJB = J // 128                   # 32 j-blocks
NIT = N // 512                   # 4 i-blocks
NDB = DIM // 128                 # 8 D-blocks

REPS = 1                         # test-only: on-device repeat count for timing
UNROLL = False                   # test-only: python-unroll reps (for TimelineSim)


def build_program_v(reps=1, unroll=False):
    global REPS, UNROLL
    old = (REPS, UNROLL)
    REPS, UNROLL = reps, unroll
    try:
        return build_program()
    finally:
        REPS, UNROLL = old


def _njb(it):
    return 4 * it + 20


def _off(it, jb):
    return 128 * jb - MEM - 512 * it


def build_program():
    nc = bacc.Bacc("TRN2", target_bir_lowering=False, debug=False,
                   num_devices=NCORES)
    ctxT_d = nc.dram_tensor("ctxT", [DIM, J], BF16, kind="ExternalInput").ap()
    wq_d = nc.dram_tensor("wq", [DIM, DHC], BF16, kind="ExternalInput").ap()
    wk_d = nc.dram_tensor("wk", [DIM, DHC], BF16, kind="ExternalInput").ap()
    wv_d = nc.dram_tensor("wv", [DIM, DHC], BF16, kind="ExternalInput").ap()
    wo_d = nc.dram_tensor("wo", [DHC, DIM], BF16, kind="ExternalInput").ap()
    exp_d = nc.dram_tensor("expire", [NJB, 128], F32, kind="ExternalInput").ap()
    msk_d = nc.dram_tensor("masks", [4, 128, 512], BF16, kind="ExternalInput").ap()
    out_d = nc.dram_tensor("out", [N, DIM], F32, kind="ExternalOutput").ap()

    with tile.TileContext(nc) as tc, ExitStack() as ctx:
        sb = ctx.enter_context(tc.tile_pool(name="sb", bufs=1))
        pb = ctx.enter_context(tc.tile_pool(name="pb", bufs=1))
        ob = ctx.enter_context(tc.tile_pool(name="ob", bufs=1))
        pp = ctx.enter_context(tc.tile_pool(name="pp", bufs=1, space="PSUM"))
        dp = ctx.enter_context(tc.tile_pool(name="dp", bufs=2, space="DRAM"))

        # ---- constants / small inputs ----
        expire = sb.tile([128, NJB], F32)
        masks = sb.tile([128, 4, 512], BF16)
        wq = sb.tile([128, NDB, DHC], BF16)
        wk = sb.tile([128, NDB, DHC], BF16)
        wv = sb.tile([128, NDB, DHC], BF16)
        cx = sb.tile([128, NDB, J], BF16)
        wo = sb.tile([128, 2, DIM], BF16)

        # DMA in first-use order, split across the two HWDGE queues (sync +
        # scalar) so descriptor-ring issue (~0.8us each) doesn't serialize:
        # sync: wq + it0 q-chunk (unblocks the first matmul), k/v weights.
        # scalar (idle until the first exp): the big ctx waves, masks, wo.
        nc.sync.dma_start(out=wq, in_=wq_d.rearrange("(db p) m -> p db m", p=128))
        nc.scalar.dma_start(out=wk, in_=wk_d.rearrange("(db p) m -> p db m", p=128))

        # augmented v': per head 64 v-dims * expire + ones column (denom)
        ones64 = sb.tile([1, 64], BF16)
        nc.vector.memset(ones64, 1.0)
        warm_r = sb.tile([1, 512], BF16)
        nc.vector.memset(warm_r, 1.0)
        vpa = sb.tile([128, NJB, HPC, DH + 1], BF16)
        nc.vector.memset(vpa[:, :, :, DH:DH + 1], 1.0)

        qT = [sb.tile([128, N], BF16, name=f"qT{p}", tag=f"qT{p}") for p in range(2)]
        kT = [sb.tile([128, J], BF16, name=f"kT{p}", tag=f"kT{p}") for p in range(2)]
        ao = [sb.tile([128, N], BF16, name=f"ao{p}", tag=f"ao{p}") for p in range(2)]

        # keep the PE busy while the first DMAs land so the HAM clock-gate
        # reaches 8/8 before the first real matmul (and stays there)
        warm_ps = pp.tile([128, 1024], F32, name="warm", tag="s", bufs=2)

        def warm(n, cols=128):
            for _w in range(n):
                nc.tensor.matmul(warm_ps[0:64, 0:cols], lhsT=ones64,
                                 rhs=warm_r[:, 0:cols],
                                 start=True, stop=True, skip_group_check=True)

        warm(40)

        rep_cm = tc.For_i(0, REPS, 1) if REPS > 1 and not UNROLL else None
        if rep_cm is not None:
            rep_cm.__enter__()

        # ---- context load, ordered by first use and split across the two
        # HWDGE queues (~268GB/s each). sync: q-chunk, wv, mem jt2/3, x rest;
        # scalar: mem jt0/jt1 (rings done before the first exp needs ACT).
        for _rep in range(REPS if UNROLL else 1):
            for db in range(NDB):
                nc.sync.dma_start(out=cx[:, db, MEM:MEM + 512],
                                  in_=ctxT_d[128 * db:128 * db + 128, MEM:MEM + 512])
            for db in range(NDB):
                nc.scalar.dma_start(out=cx[:, db, 0:512],
                                    in_=ctxT_d[128 * db:128 * db + 128, 0:512])
            nc.sync.dma_start(out=wv, in_=wv_d.rearrange("(db p) m -> p db m", p=128))
            nc.sync.dma_start(out=expire, in_=exp_d.rearrange("j p -> p j"))
            for db in range(NDB):
                nc.scalar.dma_start(out=cx[:, db, 512:1024],
                                    in_=ctxT_d[128 * db:128 * db + 128, 512:1024])
            for db in range(NDB):
                nc.sync.dma_start(out=cx[:, db, 1024:MEM],
                                  in_=ctxT_d[128 * db:128 * db + 128, 1024:MEM])
            nc.sync.dma_start(out=masks, in_=msk_d.rearrange("o p i -> p o i"))
            nc.sync.dma_start(out=wo, in_=wo_d.rearrange("(pr p) m -> p pr m", p=128))
            for db in range(NDB):
                nc.sync.dma_start(out=cx[:, db, MEM + 512:J],
                                  in_=ctxT_d[128 * db:128 * db + 128, MEM + 512:J])

            def proj_k(jt, pr):
                ps = pp.tile([128, 1024], F32, name="ps", tag="s", bufs=2)
                for db in range(NDB):
                    nc.tensor.matmul(
                        ps[:, 0:512], lhsT=wk[:, db, 128 * pr:128 * pr + 128],
                        rhs=cx[:, db, 512 * jt:512 * jt + 512],
                        start=(db == 0), stop=(db == NDB - 1))
                nc.vector.tensor_copy(out=kT[pr][:, 512 * jt:512 * jt + 512],
                                      in_=ps[:, 0:512])

            def proj_q(it, pr):
                ps = pp.tile([128, 1024], F32, name="ps", tag="s", bufs=2)
                for db in range(NDB):
                    nc.tensor.matmul(
                        ps[:, 0:512], lhsT=wq[:, db, 128 * pr:128 * pr + 128],
                        rhs=cx[:, db, MEM + 512 * it:MEM + 512 * it + 512],
                        start=(db == 0), stop=(db == NDB - 1))
                nc.vector.tensor_copy(out=qT[pr][:, 512 * it:512 * it + 512],
                                      in_=ps[:, 0:512])

            def proj_v(jb):
                ps = pp.tile([128, 1024], F32, name="ps", tag="s", bufs=2)
                for db in range(NDB):
                    nc.tensor.matmul(
                        ps[:, 0:DHC], lhsT=cx[:, db, 128 * jb:128 * jb + 128],
                        rhs=wv[:, db, :],
                        start=(db == 0), stop=(db == NDB - 1))
                nc.vector.tensor_scalar(
                    out=vpa[:, jb, :, 0:DH],
                    in0=ps[:, 0:DHC].rearrange("p (h d) -> p h d", h=HPC),
                    scalar1=expire[:, jb:jb + 1], scalar2=None, op0=MULT)

            ot_hold = {}

            def outproj_ib(ib):
                # one matmul+copy per ib; quad out-DMA fires on ib%4==3
                ps = pp.tile([128, 1024], F32, name="ps_o", tag="s", bufs=2)
                for nb in range(2):
                    for pr in range(2):
                        nc.tensor.matmul(
                            ps[:, 512 * nb:512 * nb + 512],
                            lhsT=ao[pr][:, 128 * ib:128 * ib + 128],
                            rhs=wo[:, pr, 512 * nb:512 * nb + 512],
                            start=(pr == 0), stop=(pr == 1))
                half = ib % 4
                if half == 0:
                    ot_hold["t"] = ob.tile([128, 4, 1024], F32, name="ot",
                                           tag="ot", bufs=2)
                ot4 = ot_hold["t"]
                nc.vector.tensor_copy(out=ot4[:, half, :], in_=ps)
                if half == 3:
                    ib0 = ib - 3
                    nc.sync.dma_start(
                        out=out_d[128 * ib0:128 * ib0 + 512, :].rearrange(
                            "(i p) n -> p i n", p=128),
                        in_=ot4)

            def emit_pv_group(pvd, prevmap, is_first, is_last, heads):
                pv = pvd
                for h in heads:
                    p_t, jb0, pit = prevmap[h]
                    for half, jb in enumerate((jb0, jb0 + 1)):
                        off = _off(pit, jb)
                        lo = off if 0 <= off < 512 else 0
                        nc.tensor.matmul(
                            pv[h][0:DH + 1, lo:512],
                            lhsT=vpa[:, jb, h, :],
                            rhs=p_t[:, 512 * half + lo:512 * half + 512],
                            start=(is_first and half == 0),
                            stop=(is_last and half == 1),
                            skip_group_check=True)

            def finalize_pr(pvd, isl, pr):
                # denominator rows (bf16, ACT copy: PSUM-close, no table
                # switch) -> broadcast l to 64 rows via contract-1 PE matmul
                # -> one full-lane approx reciprocal -> scale.
                pv = pvd
                lsb = [ob.tile([1, 512], BF16, name=f"lsb{e}", tag=f"lsb{e}",
                               bufs=2) for e in range(2)]
                with nc.allow_low_precision(reason="1/l broadcast in bf16"):
                    for e in range(2):
                        h = 2 * pr + e
                        nc.vector.tensor_copy(out=lsb[e],
                                              in_=pv[h][DH:DH + 1, :])
                bc_ps = pp.tile([128, 1024], F32, name="bc_ps", tag="s", bufs=2)
                for e in range(2):
                    nc.tensor.matmul(bc_ps[64 * e:64 * e + 64, 0:512],
                                     lhsT=ones64, rhs=lsb[e],
                                     start=True, stop=True,
                                     tile_position=(0, 64 * e),
                                     skip_group_check=True)
                bc = ob.tile([128, 512], F32, name="bc", tag="bc", bufs=2)
                nc.vector.reciprocal_approx_fast(out=bc, in_=bc_ps[:, 0:512])
                for e in range(2):
                    h = 2 * pr + e
                    nc.vector.tensor_tensor(ao[pr][64 * e:64 * e + 64, isl],
                                            pv[h][0:DH, :],
                                            bc[64 * e:64 * e + 64, :], MULT)

            def run_pending(pending, n=2):
                for _ in range(min(n, len(pending))):
                    kind, arg = pending.pop(0)
                    if kind == "k":
                        proj_k(*arg)
                    elif kind == "v":
                        proj_v(arg)
                    elif kind == "q":
                        proj_q(*arg)
                    elif kind == "o":
                        outproj_ib(arg)
                    elif kind == "fp":
                        emit_pv_group(*arg)
                    elif kind == "fin":
                        finalize_pr(*arg)
                    elif kind == "w":
                        warm(arg, 512)

            # ---- prologue: bare minimum for S at it0 jj0 pr0; pr1's q/k
            # are the first pending pops (emitted between pr0 and pr1
            # sections of jj0), so the first exp starts ~10us earlier.
            proj_q(0, 0)
            proj_k(0, 0)

            carry = None   # prev it's (pv, prevmap, first_flag, isl) awaiting PV+finalize
            for it in range(NIT):
                njb = _njb(it)
                npair = njb // 2
                i0 = 512 * it
                isl = slice(i0, i0 + 512)

                pending = []
                if carry is not None:
                    # real PE work (k-proj) between the carry PV and each
                    # finalize fills the ACT-copy/approx latency; the "w"
                    # bundles are backstop so the HAM clock never re-gates
                    cpv, cprev, cfirst, cisl = carry
                    pending += [("fp", (cpv, cprev, cfirst, True, range(HPC))),
                                ("k", (4 + it, 0)),
                                ("fin", (cpv, cisl, 0)),
                                ("k", (4 + it, 1)),
                                ("fin", (cpv, cisl, 1))]
                if it == 0:
                    # deadlines: k jt (both prs) before S at jj=2*jt; v jb
                    # before PV at jj=jb//2+1 (PV lags one jj). Four pops per
                    # jj (two per pr section) meet these comfortably. q(1) is
                    # last (x-chunk DMA lands late).
                    pending += [("q", (0, 1)), ("k", (0, 1)),
                                ("v", 0), ("v", 1), ("v", 2), ("v", 3)]
                    for jt in range(1, 5):
                        pending += [("k", (jt, 0)), ("k", (jt, 1)),
                                    ("v", 2 * jt + 2), ("v", 2 * jt + 3)]
                    pending += [("v", jb) for jb in range(12, 20)]
                    pending += [("q", (1, 0)), ("q", (1, 1))]
                else:
                    pending += [("v", 16 + 4 * it), ("v", 17 + 4 * it),
                                ("v", 18 + 4 * it), ("v", 19 + 4 * it)]
                    if it < NIT - 1:
                        pending += [("q", (it + 1, 0)), ("q", (it + 1, 1))]
                    pending += [("o", ib) for ib in range(4 * (it - 1), 4 * it)]

                pvd = [pp.tile([128, 512], F32, name=f"pv{h}", tag=f"pv{h}",
                               bufs=1) for h in range(HPC)]

                prev = None
                for jj in range(npair):
                    jb0 = 2 * jj
                    # diagonal tiles: columns below `off` are fully masked —
                    # S/exp/mask/PV all restrict to the live column range
                    off0 = _off(it, jb0)
                    lo0 = off0 if 0 <= off0 < 512 else 0
                    cur = {}
                    for pr in range(2):
                        s_h = [pp.tile([128, 1024], F32, name=f"s{e}", tag="s",
                                       bufs=2) for e in range(2)]
                        for half, jb in enumerate((jb0, jb0 + 1)):
                            off = _off(it, jb)
                            lo = off if 0 <= off < 512 else 0
                            jsl = slice(128 * jb, 128 * jb + 128)
                            fsl = slice(512 * half + lo, 512 * half + 512)
                            qsl = slice(i0 + lo, i0 + 512)
                            nc.tensor.matmul(s_h[0][:, fsl], lhsT=kT[pr][0:64, jsl],
                                             rhs=qT[pr][0:64, qsl],
                                             start=True, stop=True, tile_position=(0, 0))
                            nc.tensor.matmul(s_h[1][:, fsl], lhsT=kT[pr][64:128, jsl],
                                             rhs=qT[pr][64:128, qsl],
                                             start=True, stop=True, tile_position=(64, 0))
                        for e in range(2):
                            h = 2 * pr + e
                            p_t = pb.tile([128, 1024], BF16, name="p_t", tag="p", bufs=8)
                            nc.scalar.activation(p_t[:, lo0:1024], s_h[e][:, lo0:1024],
                                                 AF.Exp, scale=SCALE)
                            for half, jb in enumerate((jb0, jb0 + 1)):
                                off = _off(it, jb)
                                if 0 <= off < 512:
                                    msl = slice(512 * half + off,
                                                512 * half + off + 128)
                                    nc.vector.tensor_tensor(
                                        p_t[:, msl], p_t[:, msl],
                                        masks[:, off // 128, off:off + 128], MULT)
                            cur[h] = (p_t, jb0, it)
                        # after S of this pr: PV of previous jj (same heads)
                        if prev is not None:
                            emit_pv_group(pvd, prev, jj == 1, False,
                                          (2 * pr, 2 * pr + 1))
                        if it == 0 or jj < 2:
                            run_pending(pending, 2)
                        elif pr == 0:
                            run_pending(pending, 1)
                    prev = cur
                run_pending(pending, len(pending))
                carry = (pvd, prev, npair == 1, isl)

            # last it: pr1's PV+finalize first so its outproj partials can
            # start while pr0's finalize chain drains; copies split across
            # ACT+DVE; per-ib DMAs fire as each copy lands
            cpv, cprev, cfirst, cisl = carry
            emit_pv_group(cpv, cprev, cfirst, True, range(HPC))
            finalize_pr(cpv, cisl, 0)
            finalize_pr(cpv, cisl, 1)

            qb = 4 * (NIT - 1)
            ot4 = ob.tile([128, 4, 1024], F32, name="ot", tag="ot", bufs=2)
            for p2 in range(2):
                pss = []
                for ib in (qb + 2 * p2, qb + 2 * p2 + 1):
                    ps = pp.tile([128, 1024], F32, name="ps_o", tag="s", bufs=2)
                    pss.append((ib, ps))
                for pr in range(2):
                    for ib, ps in pss:
                        for nb in range(2):
                            nc.tensor.matmul(
                                ps[:, 512 * nb:512 * nb + 512],
                                lhsT=ao[pr][:, 128 * ib:128 * ib + 128],
                                rhs=wo[:, pr, 512 * nb:512 * nb + 512],
                                start=(pr == 0), stop=(pr == 1))
                for idx, (ib, ps) in enumerate(pss):
                    if idx == 0:
                        nc.scalar.activation(ot4[:, ib - qb, :], ps, AF.Copy)
                    else:
                        nc.vector.tensor_copy(out=ot4[:, ib - qb, :], in_=ps)
                    # alternate HWDGE queues so the four 512KB tail
                    # transfers run two-abreast instead of serializing
                    eng = nc.sync if idx == 0 else nc.scalar
                    eng.dma_start(
                        out=out_d[128 * ib:128 * ib + 128, :],
                        in_=ot4[:, ib - qb, :])
        if rep_cm is not None:
            rep_cm.__exit__(None, None, None)
    nc.compile()
    return nc


_NC = None


def _get_nc():
    global _NC
    if _NC is None:
        _NC = build_program()
    return _NC


def _make_masks():
    m = np.zeros((4, 128, 512), dtype=ml_dtypes.bfloat16)
    fi = np.arange(512)[None, :]
    fj = np.arange(128)[:, None]
    for o in range(4):
        m[o] = (fi >= fj + 128 * o).astype(ml_dtypes.bfloat16)
    return m


def make_in_maps(x, mem, expire_mask, Wq, Wkv, Wo):
    bf = ml_dtypes.bfloat16
    masks = _make_masks()
    ctxT = []
    for b in range(B):
        c = np.concatenate([mem[b], x[b]], axis=0)          # [J, DIM]
        ctxT.append(np.ascontiguousarray(c.T).astype(bf))   # [DIM, J]

    in_maps = []
    for core in range(NCORES):
        b, hg = core // 4, core % 4
        cs = slice(DHC * hg, DHC * hg + DHC)
        in_maps.append({
            "ctxT": ctxT[b],
            "wq": np.ascontiguousarray(Wq[:, cs]).astype(bf),
            "wk": np.ascontiguousarray(Wkv[:, cs]).astype(bf),
            "wv": np.ascontiguousarray(Wkv[:, DIM + cs.start:DIM + cs.stop]).astype(bf),
            "wo": np.ascontiguousarray(Wo[cs, :]).astype(bf),
            "expire": np.ascontiguousarray(expire_mask[b, 0, 0].reshape(NJB, 128)),
            "masks": masks,
        })
    return in_maps


def kernel(x, mem, expire_mask, Wq, Wkv, Wo, bo):
    x = np.asarray(x, dtype=np.float32)
    mem = np.asarray(mem, dtype=np.float32)
    expire_mask = np.asarray(expire_mask, dtype=np.float32)
    Wq = np.asarray(Wq, dtype=np.float32)
    Wkv = np.asarray(Wkv, dtype=np.float32)
    Wo = np.asarray(Wo, dtype=np.float32)
    bo = np.asarray(bo, dtype=np.float32)

    in_maps = make_in_maps(x, mem, expire_mask, Wq, Wkv, Wo)
    nc = _get_nc()
    res = run_bass_kernel_spmd(nc, in_maps, core_ids=list(range(NCORES)))

    out = np.zeros((B, N, DIM), dtype=np.float32)
    for core in range(NCORES):
        out[core // 4] += res.results[core]["out"].astype(np.float32)
    out += bo[None, None, :]
    return out



# revision 50
# speedup vs baseline: 1.1892x; 1.0037x over previous
"""Causal attention with memory + post-softmax expire gating, on 8 trn2 cores.

Sharding: batch (2) x head-groups (4 heads each) -> 8 cores. Each core
computes q/k/v projections for its 4 heads (column-parallel), local
attention, and a partial output projection (row-parallel over heads).
Host sums the 4 partial products per batch and adds the bias.

v3: software-pipelined emission.
  - softmax denominator rides the PV matmul as a 65th stationary column.
  - ctx DMA: q's it0 column chunk first, then mem half, then the rest.
  - projections / output projection spread one item per jj iteration.
  - PV lags S by one jj so PE has independent work while ACT drains.

v12 (~356us, from 419us):
  - DMAs split across sync+scalar HWDGE queues (~268GB/s each), per-db
    rings so projection db-accumulation paces with chunk arrivals.
  - warm-up matmuls cover the DMA-bound start so the HAM clock-gate
    (1.2->2.4GHz) warms early and never re-gates (throttle 96us -> 27us).
  - minimal prologue (q00/k00 only; q01/k01/v* ride the pending queue):
    first exp at ~29us instead of ~61us.
  - finalize chain: ACT-copy denominator rows (no table switch), bf16
    ones-broadcast matmul, one full-lane reciprocal_approx_fast --
    replaces 16x 3.3us single-lane DVE reciprocals; kills the ~9us
    PE stall + HAM re-gate at every i-block boundary.
  - diagonal tiles: S/exp/mask/PV restricted to live columns.
  - k-projection filler between carry-PV and finalizes at boundaries.
  - tail: per-ib out-DMAs, copies split ACT||DVE.

v13 (~355us): finalize lsb copies moved ACT->DVE (ACT gates the late
  jjs of it1-3 where per-jj exp 4.6us > per-jj S+PV 3.1us); pending
  pops spread (2/section for jj<2, then 1) so filler lasts the loop.
  Measured (NTFF, min-of-3; board clock varies ~20% run-to-run):
  PE busy ~326us (gapless), ACT ~222us, DVE ~92us.
  Also dead: bf16 output partials (DMA halves but cast/granularity
  costs more, +0.2e-3 err for +3us).
  Dead ends (hardware-measured): PV col-tile head-packing pairs stream
  1.71x but separate denominator matmuls eat the entire gain; M=1
  denom quads ~99ns/MM still net-negative.
"""

import numpy as np
import ml_dtypes
from contextlib import ExitStack

import concourse.bass as bass
import concourse.mybir as mybir
import concourse.tile as tile
from concourse import bacc
from concourse.bass_utils import run_bass_kernel_spmd

F32 = mybir.dt.float32
BF16 = mybir.dt.bfloat16
AF = mybir.ActivationFunctionType
MULT = mybir.AluOpType.mult

HEADS = 16
B, N, MEM, DIM = 2, 2048, 2048, 1024
J = MEM + N                      # 4096
DH = 64                          # head dim
HPC = 4                          # heads per core
DHC = HPC * DH                   # 256 dims per core
SCALE = DH ** -0.5
NCORES = 8

NJB = J // 128                   # 32 j-blocks
NIT = N // 512                   # 4 i-blocks
NDB = DIM // 128                 # 8 D-blocks

REPS = 1                         # test-only: on-device repeat count for timing
UNROLL = False                   # test-only: python-unroll reps (for TimelineSim)


def build_program_v(reps=1, unroll=False):
    global REPS, UNROLL
    old = (REPS, UNROLL)
    REPS, UNROLL = reps, unroll
    try:
        return build_program()
    finally:
        REPS, UNROLL = old


def _njb(it):
    return 4 * it + 20


def _off(it, jb):
    return 128 * jb - MEM - 512 * it


def build_program():
    nc = bacc.Bacc("TRN2", target_bir_lowering=False, debug=False,
                   num_devices=NCORES)
    ctxT_d = nc.dram_tensor("ctxT", [DIM, J], BF16, kind="ExternalInput").ap()
    wq_d = nc.dram_tensor("wq", [DIM, DHC], BF16, kind="ExternalInput").ap()
    wk_d = nc.dram_tensor("wk", [DIM, DHC], BF16, kind="ExternalInput").ap()
    wv_d = nc.dram_tensor("wv", [DIM, DHC], BF16, kind="ExternalInput").ap()
    wo_d = nc.dram_tensor("wo", [DHC, DIM], BF16, kind="ExternalInput").ap()
    exp_d = nc.dram_tensor("expire", [NJB, 128], F32, kind="ExternalInput").ap()
    msk_d = nc.dram_tensor("masks", [4, 128, 512], BF16, kind="ExternalInput").ap()
    out_d = nc.dram_tensor("out", [N, DIM], F32, kind="ExternalOutput").ap()

    with tile.TileContext(nc) as tc, ExitStack() as ctx:
        sb = ctx.enter_context(tc.tile_pool(name="sb", bufs=1))
        pb = ctx.enter_context(tc.tile_pool(name="pb", bufs=1))
        ob = ctx.enter_context(tc.tile_pool(name="ob", bufs=1))
        pp = ctx.enter_context(tc.tile_pool(name="pp", bufs=1, space="PSUM"))
        dp = ctx.enter_context(tc.tile_pool(name="dp", bufs=2, space="DRAM"))

        # ---- constants / small inputs ----
        expire = sb.tile([128, NJB], F32)
        masks = sb.tile([128, 4, 512], BF16)
        wq = sb.tile([128, NDB, DHC], BF16)
        wk = sb.tile([128, NDB, DHC], BF16)
        wv = sb.tile([128, NDB, DHC], BF16)
        cx = sb.tile([128, NDB, J], BF16)
        wo = sb.tile([128, 2, DIM], BF16)

        # DMA in first-use order, split across the two HWDGE queues (sync +
        # scalar) so descriptor-ring issue (~0.8us each) doesn't serialize:
        # sync: wq + it0 q-chunk (unblocks the first matmul), k/v weights.
        # scalar (idle until the first exp): the big ctx waves, masks, wo.
        # pr0 halves first: q00/k00 need only columns 0:128 of wq/wk, so
        # the q-chunk/mem waves start ~1us earlier; pr1 halves follow the
        # first ctx wave (consumed via pending pops much later)
        nc.sync.dma_start(out=wq[:, :, 0:128],
                          in_=wq_d[:, 0:128].rearrange("(db p) m -> p db m", p=128))
        nc.scalar.dma_start(out=wk[:, :, 0:128],
                            in_=wk_d[:, 0:128].rearrange("(db p) m -> p db m", p=128))

        # augmented v': per head 64 v-dims * expire + ones column (denom)
        ones64 = sb.tile([1, 64], BF16)
        nc.vector.memset(ones64, 1.0)
        warm_r = sb.tile([1, 512], BF16)
        nc.vector.memset(warm_r, 1.0)
        vpa = sb.tile([128, NJB, HPC, DH + 1], BF16)
        nc.vector.memset(vpa[:, :, :, DH:DH + 1], 1.0)

        qT = [sb.tile([128, N], BF16, name=f"qT{p}", tag=f"qT{p}") for p in range(2)]
        kT = [sb.tile([128, J], BF16, name=f"kT{p}", tag=f"kT{p}") for p in range(2)]
        ao = [sb.tile([128, N], BF16, name=f"ao{p}", tag=f"ao{p}") for p in range(2)]

        # keep the PE busy while the first DMAs land so the HAM clock-gate
        # reaches 8/8 before the first real matmul (and stays there)
        warm_ps = pp.tile([128, 1024], F32, name="warm", tag="s", bufs=2)

        def warm(n, cols=128):
            for _w in range(n):
                nc.tensor.matmul(warm_ps[0:64, 0:cols], lhsT=ones64,
                                 rhs=warm_r[:, 0:cols],
                                 start=True, stop=True, skip_group_check=True)

        warm(40)

        rep_cm = tc.For_i(0, REPS, 1) if REPS > 1 and not UNROLL else None
        if rep_cm is not None:
            rep_cm.__enter__()

        # ---- context load, ordered by first use and split across the two
        # HWDGE queues (~268GB/s each). sync: q-chunk, wv, mem jt2/3, x rest;
        # scalar: mem jt0/jt1 (rings done before the first exp needs ACT).
        for _rep in range(REPS if UNROLL else 1):
            for db in range(NDB):
                nc.sync.dma_start(out=cx[:, db, MEM:MEM + 512],
                                  in_=ctxT_d[128 * db:128 * db + 128, MEM:MEM + 512])
            for db in range(NDB):
                nc.scalar.dma_start(out=cx[:, db, 0:512],
                                    in_=ctxT_d[128 * db:128 * db + 128, 0:512])
            nc.sync.dma_start(out=wq[:, :, 128:256],
                              in_=wq_d[:, 128:256].rearrange("(db p) m -> p db m",
                                                             p=128))
            nc.sync.dma_start(out=wv, in_=wv_d.rearrange("(db p) m -> p db m", p=128))
            nc.sync.dma_start(out=expire, in_=exp_d.rearrange("j p -> p j"))
            nc.scalar.dma_start(out=wk[:, :, 128:256],
                                in_=wk_d[:, 128:256].rearrange("(db p) m -> p db m",
                                                               p=128))
            for db in range(NDB):
                nc.scalar.dma_start(out=cx[:, db, 512:1024],
                                    in_=ctxT_d[128 * db:128 * db + 128, 512:1024])
            for db in range(NDB):
                nc.sync.dma_start(out=cx[:, db, 1024:MEM],
                                  in_=ctxT_d[128 * db:128 * db + 128, 1024:MEM])
            nc.sync.dma_start(out=masks, in_=msk_d.rearrange("o p i -> p o i"))
            nc.sync.dma_start(out=wo, in_=wo_d.rearrange("(pr p) m -> p pr m", p=128))
            for db in range(NDB):
                nc.sync.dma_start(out=cx[:, db, MEM + 512:J],
                                  in_=ctxT_d[128 * db:128 * db + 128, MEM + 512:J])

            def proj_k(jt, pr):
                ps = pp.tile([128, 1024], F32, name="ps", tag="s", bufs=2)
                for db in range(NDB):
                    nc.tensor.matmul(
                        ps[:, 0:512], lhsT=wk[:, db, 128 * pr:128 * pr + 128],
                        rhs=cx[:, db, 512 * jt:512 * jt + 512],
                        start=(db == 0), stop=(db == NDB - 1))
                nc.vector.tensor_copy(out=kT[pr][:, 512 * jt:512 * jt + 512],
                                      in_=ps[:, 0:512])

            def proj_q(it, pr):
                ps = pp.tile([128, 1024], F32, name="ps", tag="s", bufs=2)
                for db in range(NDB):
                    nc.tensor.matmul(
                        ps[:, 0:512], lhsT=wq[:, db, 128 * pr:128 * pr + 128],
                        rhs=cx[:, db, MEM + 512 * it:MEM + 512 * it + 512],
                        start=(db == 0), stop=(db == NDB - 1))
                nc.vector.tensor_copy(out=qT[pr][:, 512 * it:512 * it + 512],
                                      in_=ps[:, 0:512])

            def proj_v(jb):
                ps = pp.tile([128, 1024], F32, name="ps", tag="s", bufs=2)
                for db in range(NDB):
                    nc.tensor.matmul(
                        ps[:, 0:DHC], lhsT=cx[:, db, 128 * jb:128 * jb + 128],
                        rhs=wv[:, db, :],
                        start=(db == 0), stop=(db == NDB - 1))
                nc.vector.tensor_scalar(
                    out=vpa[:, jb, :, 0:DH],
                    in0=ps[:, 0:DHC].rearrange("p (h d) -> p h d", h=HPC),
                    scalar1=expire[:, jb:jb + 1], scalar2=None, op0=MULT)

            ot_hold = {}

            def outproj_ib(ib):
                # one matmul+copy per ib; quad out-DMA fires on ib%4==3
                ps = pp.tile([128, 1024], F32, name="ps_o", tag="s", bufs=2)
                for nb in range(2):
                    for pr in range(2):
                        nc.tensor.matmul(
                            ps[:, 512 * nb:512 * nb + 512],
                            lhsT=ao[pr][:, 128 * ib:128 * ib + 128],
                            rhs=wo[:, pr, 512 * nb:512 * nb + 512],
                            start=(pr == 0), stop=(pr == 1))
                half = ib % 4
                if half == 0:
                    ot_hold["t"] = ob.tile([128, 4, 1024], F32, name="ot",
                                           tag="ot", bufs=2)
                ot4 = ot_hold["t"]
                nc.vector.tensor_copy(out=ot4[:, half, :], in_=ps)
                if half == 3:
                    ib0 = ib - 3
                    nc.sync.dma_start(
                        out=out_d[128 * ib0:128 * ib0 + 512, :].rearrange(
                            "(i p) n -> p i n", p=128),
                        in_=ot4)

            def emit_pv_group(pvd, prevmap, is_first, is_last, heads):
                pv = pvd
                for h in heads:
                    p_t, jb0, pit = prevmap[h]
                    for half, jb in enumerate((jb0, jb0 + 1)):
                        off = _off(pit, jb)
                        lo = off if 0 <= off < 512 else 0
                        nc.tensor.matmul(
                            pv[h][0:DH + 1, lo:512],
                            lhsT=vpa[:, jb, h, :],
                            rhs=p_t[:, 512 * half + lo:512 * half + 512],
                            start=(is_first and half == 0),
                            stop=(is_last and half == 1),
                            skip_group_check=True)

            def finalize_pr(pvd, isl, pr):
                # denominator rows (bf16, ACT copy: PSUM-close, no table
                # switch) -> broadcast l to 64 rows via contract-1 PE matmul
                # -> one full-lane approx reciprocal -> scale.
                pv = pvd
                lsb = [ob.tile([1, 512], BF16, name=f"lsb{e}", tag=f"lsb{e}",
                               bufs=2) for e in range(2)]
                with nc.allow_low_precision(reason="1/l broadcast in bf16"):
                    for e in range(2):
                        h = 2 * pr + e
                        nc.vector.tensor_copy(out=lsb[e],
                                              in_=pv[h][DH:DH + 1, :])
                bc_ps = pp.tile([128, 1024], F32, name="bc_ps", tag="s", bufs=2)
                for e in range(2):
                    nc.tensor.matmul(bc_ps[64 * e:64 * e + 64, 0:512],
                                     lhsT=ones64, rhs=lsb[e],
                                     start=True, stop=True,
                                     tile_position=(0, 64 * e),
                                     skip_group_check=True)
                bc = ob.tile([128, 512], F32, name="bc", tag="bc", bufs=2)
                nc.vector.reciprocal_approx_fast(out=bc, in_=bc_ps[:, 0:512])
                for e in range(2):
                    h = 2 * pr + e
                    nc.vector.tensor_tensor(ao[pr][64 * e:64 * e + 64, isl],
                                            pv[h][0:DH, :],
                                            bc[64 * e:64 * e + 64, :], MULT)

            def run_pending(pending, n=2):
                for _ in range(min(n, len(pending))):
                    kind, arg = pending.pop(0)
                    if kind == "k":
                        proj_k(*arg)
                    elif kind == "v":
                        proj_v(arg)
                    elif kind == "q":
                        proj_q(*arg)
                    elif kind == "o":
                        outproj_ib(arg)
                    elif kind == "fp":
                        emit_pv_group(*arg)
                    elif kind == "fin":
                        finalize_pr(*arg)
                    elif kind == "w":
                        warm(arg, 512)

            # ---- prologue: bare minimum for S at it0 jj0 pr0; pr1's q/k
            # are the first pending pops (emitted between pr0 and pr1
            # sections of jj0), so the first exp starts ~10us earlier.
            proj_q(0, 0)
            proj_k(0, 0)

            carry = None   # prev it's (pv, prevmap, first_flag, isl) awaiting PV+finalize
            for it in range(NIT):
                njb = _njb(it)
                npair = njb // 2
                i0 = 512 * it
                isl = slice(i0, i0 + 512)

                pending = []
                if carry is not None:
                    # real PE work (k-proj) between the carry PV and each
                    # finalize fills the ACT-copy/approx latency; the "w"
                    # bundles are backstop so the HAM clock never re-gates
                    cpv, cprev, cfirst, cisl = carry
                    pending += [("fp", (cpv, cprev, cfirst, True, range(HPC))),
                                ("k", (4 + it, 0)),
                                ("fin", (cpv, cisl, 0)),
                                ("k", (4 + it, 1)),
                                ("fin", (cpv, cisl, 1))]
                if it == 0:
                    # deadlines: k jt (both prs) before S at jj=2*jt; v jb
                    # before PV at jj=jb//2+1 (PV lags one jj). Four pops per
                    # jj (two per pr section) meet these comfortably. q(1) is
                    # last (x-chunk DMA lands late).
                    pending += [("q", (0, 1)), ("k", (0, 1)),
                                ("v", 0), ("v", 1), ("v", 2), ("v", 3)]
                    for jt in range(1, 5):
                        pending += [("k", (jt, 0)), ("k", (jt, 1)),
                                    ("v", 2 * jt + 2), ("v", 2 * jt + 3)]
                    pending += [("v", jb) for jb in range(12, 20)]
                    pending += [("q", (1, 0)), ("q", (1, 1))]
                else:
                    pending += [("v", 16 + 4 * it), ("v", 17 + 4 * it),
                                ("v", 18 + 4 * it), ("v", 19 + 4 * it)]
                    if it < NIT - 1:
                        pending += [("q", (it + 1, 0)), ("q", (it + 1, 1))]
                    pending += [("o", ib) for ib in range(4 * (it - 1), 4 * it)]

                pvd = [pp.tile([128, 512], F32, name=f"pv{h}", tag=f"pv{h}",
                               bufs=1) for h in range(HPC)]

                prev = None
                for jj in range(npair):
                    jb0 = 2 * jj
                    # diagonal tiles: columns below `off` are fully masked —
                    # S/exp/mask/PV all restrict to the live column range
                    off0 = _off(it, jb0)
                    lo0 = off0 if 0 <= off0 < 512 else 0
                    cur = {}
                    for pr in range(2):
                        s_h = [pp.tile([128, 1024], F32, name=f"s{e}", tag="s",
                                       bufs=2) for e in range(2)]
                        for half, jb in enumerate((jb0, jb0 + 1)):
                            off = _off(it, jb)
                            lo = off if 0 <= off < 512 else 0
                            jsl = slice(128 * jb, 128 * jb + 128)
                            fsl = slice(512 * half + lo, 512 * half + 512)
                            qsl = slice(i0 + lo, i0 + 512)
                            nc.tensor.matmul(s_h[0][:, fsl], lhsT=kT[pr][0:64, jsl],
                                             rhs=qT[pr][0:64, qsl],
                                             start=True, stop=True, tile_position=(0, 0))
                            nc.tensor.matmul(s_h[1][:, fsl], lhsT=kT[pr][64:128, jsl],
                                             rhs=qT[pr][64:128, qsl],
                                             start=True, stop=True, tile_position=(64, 0))
                        for e in range(2):
                            h = 2 * pr + e
                            p_t = pb.tile([128, 1024], BF16, name="p_t", tag="p", bufs=8)
                            nc.scalar.activation(p_t[:, lo0:1024], s_h[e][:, lo0:1024],
                                                 AF.Exp, scale=SCALE)
                            for half, jb in enumerate((jb0, jb0 + 1)):
                                off = _off(it, jb)
                                if 0 <= off < 512:
                                    msl = slice(512 * half + off,
                                                512 * half + off + 128)
                                    nc.vector.tensor_tensor(
                                        p_t[:, msl], p_t[:, msl],
                                        masks[:, off // 128, off:off + 128], MULT)
                            cur[h] = (p_t, jb0, it)
                        # after S of this pr: PV of previous jj (same heads)
                        if prev is not None:
                            emit_pv_group(pvd, prev, jj == 1, False,
                                          (2 * pr, 2 * pr + 1))
                        if it == 0 or jj < 2:
                            run_pending(pending, 2)
                        elif pr == 0:
                            run_pending(pending, 1)
                    prev = cur
                run_pending(pending, len(pending))
                carry = (pvd, prev, npair == 1, isl)

            # last it: pr1's PV+finalize first so its outproj partials can
            # start while pr0's finalize chain drains; copies split across
            # ACT+DVE; per-ib DMAs fire as each copy lands
            cpv, cprev, cfirst, cisl = carry
            emit_pv_group(cpv, cprev, cfirst, True, range(HPC))
            finalize_pr(cpv, cisl, 0)
            finalize_pr(cpv, cisl, 1)

            qb = 4 * (NIT - 1)
            ot4 = ob.tile([128, 4, 1024], F32, name="ot", tag="ot", bufs=2)
            for p2 in range(2):
                pss = []
                for ib in (qb + 2 * p2, qb + 2 * p2 + 1):
                    ps = pp.tile([128, 1024], F32, name="ps_o", tag="s", bufs=2)
                    pss.append((ib, ps))
                for pr in range(2):
                    for ib, ps in pss:
                        for nb in range(2):
                            nc.tensor.matmul(
                                ps[:, 512 * nb:512 * nb + 512],
                                lhsT=ao[pr][:, 128 * ib:128 * ib + 128],
                                rhs=wo[:, pr, 512 * nb:512 * nb + 512],
                                start=(pr == 0), stop=(pr == 1))
                for idx, (ib, ps) in enumerate(pss):
                    if idx == 0:
                        nc.scalar.activation(ot4[:, ib - qb, :], ps, AF.Copy)
                    else:
                        nc.vector.tensor_copy(out=ot4[:, ib - qb, :], in_=ps)
                    # alternate HWDGE queues so the four 512KB tail
                    # transfers run two-abreast instead of serializing
                    eng = nc.sync if idx == 0 else nc.scalar
                    eng.dma_start(
                        out=out_d[128 * ib:128 * ib + 128, :],
                        in_=ot4[:, ib - qb, :])
        if rep_cm is not None:
            rep_cm.__exit__(None, None, None)
    nc.compile()
    return nc


_NC = None


def _get_nc():
    global _NC
    if _NC is None:
        _NC = build_program()
    return _NC


def _make_masks():
    m = np.zeros((4, 128, 512), dtype=ml_dtypes.bfloat16)
    fi = np.arange(512)[None, :]
    fj = np.arange(128)[:, None]
    for o in range(4):
        m[o] = (fi >= fj + 128 * o).astype(ml_dtypes.bfloat16)
    return m


def make_in_maps(x, mem, expire_mask, Wq, Wkv, Wo):
    bf = ml_dtypes.bfloat16
    masks = _make_masks()
    ctxT = []
    for b in range(B):
        c = np.concatenate([mem[b], x[b]], axis=0)          # [J, DIM]
        ctxT.append(np.ascontiguousarray(c.T).astype(bf))   # [DIM, J]

    in_maps = []
    for core in range(NCORES):
        b, hg = core // 4, core % 4
        cs = slice(DHC * hg, DHC * hg + DHC)
        in_maps.append({
            "ctxT": ctxT[b],
            "wq": np.ascontiguousarray(Wq[:, cs]).astype(bf),
            "wk": np.ascontiguousarray(Wkv[:, cs]).astype(bf),
            "wv": np.ascontiguousarray(Wkv[:, DIM + cs.start:DIM + cs.stop]).astype(bf),
            "wo": np.ascontiguousarray(Wo[cs, :]).astype(bf),
            "expire": np.ascontiguousarray(expire_mask[b, 0, 0].reshape(NJB, 128)),
            "masks": masks,
        })
    return in_maps


def kernel(x, mem, expire_mask, Wq, Wkv, Wo, bo):
    x = np.asarray(x, dtype=np.float32)
    mem = np.asarray(mem, dtype=np.float32)
    expire_mask = np.asarray(expire_mask, dtype=np.float32)
    Wq = np.asarray(Wq, dtype=np.float32)
    Wkv = np.asarray(Wkv, dtype=np.float32)
    Wo = np.asarray(Wo, dtype=np.float32)
    bo = np.asarray(bo, dtype=np.float32)

    in_maps = make_in_maps(x, mem, expire_mask, Wq, Wkv, Wo)
    nc = _get_nc()
    res = run_bass_kernel_spmd(nc, in_maps, core_ids=list(range(NCORES)))

    out = np.zeros((B, N, DIM), dtype=np.float32)
    for core in range(NCORES):
        out[core // 4] += res.results[core]["out"].astype(np.float32)
    out += bo[None, None, :]
    return out

